# revision 1
# baseline (speedup 1.0000x reference)
"""Trainium2 Bass kernel for nn_Attention_73375221285454.

Multi-head self-attention (B=4, N=2048, D=768, H=12, DH=64) with key-padding
mask, distributed over 8 NeuronCores.

Sharding: core c handles batch b = c//2 and query half qh = c%2 (1024 query
rows). Each core computes K/V for its full batch (duplicated across the pair)
and attention + output projection for its query half; the 8 outputs tile the
full (4, 2048, 768) result with no collectives.

Host marshalling per core: x[b] is transposed (xkT for keys — sorted so that
unmasked keys come first, making trailing all-masked key tiles skippable —
and xqT for the query half in natural order); the bool mask becomes float
additive/multiplicative mask tables. Attention is permutation-invariant over
keys, so sorting keys (with the mask sorted identically) is exact.

Device algorithm per core (all matmuls in float32r ~ tf32):
  V    = (xkT.T @ Wv) stored as vaug [128, 16, 12, 65] with a ones column
  K^T  = Wk.T @ xkT  -> kT [128, 6, njt*128]    (only active key tiles)
  Q^T  = Wq.T @ xqT  -> qT [128, 6, 1024]
  per head h, active key tile jt:
    S^T[j, i] = K_h^T.T @ Q_h^T                (PSUM [128, 1024])
    P^T       = exp(0.125*S^T + cmneg[j])      (ACT; cmneg=-30000 if masked)
    O^T      += vaug[jt, h].T @ P^T            (PSUM [65, 1024]; row 64 = s[i])
  attnT_h = O^T[0:64] staged unnormalized; s-rows gathered into [12, 1024];
  one batched reciprocal, then per-head rank-1 (ones (x) 1/s) via PE and an
  in-place multiply normalizes attnT.
  out     = (attnT.T @ Wo) * rm01[i] + (1 - rm01[i]) (x) uniform_row
  where uniform_row = (mean_all_keys V) @ Wo reproduces the reference's
  uniform softmax over ALL keys for fully-masked query rows.

No max-subtraction is needed: logits are ~N(0,1) (exp can't overflow), masked
keys get exp(logit - 30000) == 0 exactly, and fully-masked query rows are
replaced by uniform_row at the end.
"""

import sys

sys.path.insert(0, "/opt/trn_rl_repo")

import numpy as np

import concourse.bass as bass  # noqa: F401
import concourse.mybir as mybir
import concourse.tile as tile
from concourse.tile import add_dep_helper
from concourse import bacc
from concourse.bass_utils import run_bass_kernel_spmd

P = 128
B, N, D = 4, 2048, 768
H, DH = 12, 64
NQ = N // 2              # queries per core
DC = D // P              # 6 contraction chunks
NJT_FULL = N // P        # 16 key tiles
NIT = NQ // P            # 8 query tiles
SCALE = DH ** -0.5       # 0.125
MASK_NEG = -30000.0
SORT_KEYS = True         # sort keys so all-masked key tiles are skipped

f32 = mybir.dt.float32
f32r = mybir.dt.float32r

_BUILD_CACHE = {}


def build(njt_act: int) -> "bacc.Bacc":
    """Build the SPMD program. njt_act = number of key tiles containing any
    unmasked key; trailing all-masked tiles contribute exactly zero to both
    softmax numerator and denominator and are skipped. V/meanV still cover
    all 16 tiles (masked-query rows need the mean over ALL keys)."""
    if njt_act in _BUILD_CACHE:
        return _BUILD_CACHE[njt_act]

    nk = njt_act * P  # active key columns

    nc = bacc.Bacc()
    xkT_d = nc.declare_dram_parameter("xkT", [D, N], f32, isOutput=False)
    xqT_d = nc.declare_dram_parameter("xqT", [D, NQ], f32, isOutput=False)
    wq_d = nc.declare_dram_parameter("Wq", [D, D], f32, isOutput=False)
    wk_d = nc.declare_dram_parameter("Wk", [D, D], f32, isOutput=False)
    wv_d = nc.declare_dram_parameter("Wv", [D, D], f32, isOutput=False)
    wo_d = nc.declare_dram_parameter("Wo", [D, D], f32, isOutput=False)
    # cmnegT[p, t] = 0.0 if key (t*128+p) unmasked else -30000.0
    cmneg_d = nc.declare_dram_parameter("cmnegT", [P, NJT_FULL], f32, isOutput=False)
    # rm01T[p, t] = 1.0 if query (t*128+p) unmasked else 0.0
    rm01_d = nc.declare_dram_parameter("rm01T", [P, NIT], f32, isOutput=False)
    # rmneg_row[0, i] = 1.0 - rm01[i]
    rmneg_d = nc.declare_dram_parameter("rmneg_row", [1, NQ], f32, isOutput=False)
    out_d = nc.declare_dram_parameter("out", [NQ, D], f32, isOutput=True)

    xkT_r = xkT_d.rearrange("(c p) n -> p c n", p=P).bitcast(f32r)
    xqT_r = xqT_d.rearrange("(c p) n -> p c n", p=P).bitcast(f32r)
    wv_r = wv_d.rearrange("(c p) e -> p c e", p=P).bitcast(f32r)
    wq_r = wq_d.rearrange("(c p) e -> p c e", p=P).bitcast(f32r)
    wk_r = wk_d.rearrange("(c p) e -> p c e", p=P).bitcast(f32r)
    wo_r = wo_d.rearrange("(c p) e -> p c e", p=P).bitcast(f32r)

    with tile.TileContext(nc) as tc:
        with tc.tile_pool(name="persist", bufs=1) as persist:
            # small persistent tiles
            cmneg = persist.tile([P, NJT_FULL], f32)
            nc.sync.dma_start(out=cmneg, in_=cmneg_d.ap())
            rm01 = persist.tile([P, NIT], f32)
            nc.sync.dma_start(out=rm01, in_=rm01_d.ap())
            rmneg_row = persist.tile([1, NQ], f32r)
            nc.sync.dma_start(out=rmneg_row, in_=rmneg_d.ap().bitcast(f32r))
            ones_f = persist.tile([P, H], f32)
            nc.vector.memset(ones_f, 1.0)
            ones_r = persist.tile([P, 1], f32r)
            nc.vector.tensor_copy(ones_r, ones_f[:, 0:1])
            id1 = persist.tile([1, 1], f32)
            nc.vector.memset(id1, 1.0)

            qT = persist.tile([P, DC, NQ], f32r)
            vaug = persist.tile([P, NJT_FULL, H, DH + 2], f32r)
            kT = persist.tile([P, DC, nk], f32r)
            mvT_sb = persist.tile([P, DC], f32r)   # meanV^T (already / N)
            mv_row = persist.tile([1, D], f32)

            with tc.tile_pool(name="xk_pool", bufs=1) as xk_pool:
                # ------------- phase 1: V projection (+ meanV) -------------
                xkT = xk_pool.tile([P, DC, N], f32r)
                vproj_scope = nc.named_scope("vproj"); vproj_scope.__enter__()
                with tc.tile_pool(name="wv_pool", bufs=1) as wv_pool, \
                     tc.tile_pool(name="psp2", bufs=2, space="PSUM") as psp2, \
                     tc.tile_pool(name="psmv", bufs=1, space="PSUM") as psmv:
                    wv_sb = wv_pool.tile([P, DC, D], f32r)
                    # chunked loads so the first V matmuls start early
                    for dc in range(DC):
                        nc.sync.dma_start(out=wv_sb[:, dc, :], in_=wv_r[:, dc, :])
                    for cg in range(4):
                        for dc in range(DC):
                            nc.sync.dma_start(
                                out=xkT[:, dc, cg * 512 : (cg + 1) * 512],
                                in_=xkT_r[:, dc, cg * 512 : (cg + 1) * 512],
                            )
                    for jt in range(NJT_FULL):
                        psv = psp2.tile([P, D], f32, tag="psv")
                        for dc in range(DC):
                            nc.tensor.matmul(
                                psv[:, 0:512],
                                xkT[:, dc, jt * P : (jt + 1) * P],
                                wv_sb[:, dc, 0:512],
                                start=(dc == 0),
                                stop=(dc == DC - 1),
                            )
                        for dc in range(DC):
                            nc.tensor.matmul(
                                psv[:, 512:768],
                                xkT[:, dc, jt * P : (jt + 1) * P],
                                wv_sb[:, dc, 512:768],
                                start=(dc == 0),
                                stop=(dc == DC - 1),
                            )
                        nc.vector.tensor_copy(
                            vaug[:, jt, :, 0:DH],
                            psv.rearrange("p (h d) -> p h d", h=H),
                        )
                        nc.vector.tensor_copy(
                            vaug[:, jt, :, DH : DH + 2],
                            ones_f[:, :, None].to_broadcast([P, H, 2]),
                        )

                    # meanV over ALL keys -> mvT_sb [128, 6], scaled by 1/N
                    ps_mv = psmv.tile([1, D], f32, tag="ps_mv")
                    for jt in range(NJT_FULL):
                        nc.tensor.matmul(
                            ps_mv[:, 0:512],
                            ones_r,
                            vaug[:, jt, 0:8, 0:DH],
                            start=(jt == 0),
                            stop=(jt == NJT_FULL - 1),
                        )
                    for jt in range(NJT_FULL):
                        nc.tensor.matmul(
                            ps_mv[:, 512:768],
                            ones_r,
                            vaug[:, jt, 8:12, 0:DH],
                            start=(jt == 0),
                            stop=(jt == NJT_FULL - 1),
                        )
                    nc.vector.tensor_scalar_mul(mv_row, in0=ps_mv, scalar1=1.0 / N)
                    ps_mvt = psmv.tile([P, DC], f32, tag="ps_mvt")
                    for c in range(DC):
                        nc.tensor.transpose(
                            ps_mvt[:, c : c + 1],
                            mv_row[0:1, c * P : (c + 1) * P],
                            id1,
                        )
                    nc.vector.tensor_copy(mvT_sb, ps_mvt)

                vproj_scope.__exit__(None, None, None)
                qproj_scope = nc.named_scope("qproj"); qproj_scope.__enter__()
                # ---------------- phase 2: Q projection ----------------
                with tc.tile_pool(name="xq_pool", bufs=1) as xq_pool, \
                     tc.tile_pool(name="wst1", bufs=2) as wst1, \
                     tc.tile_pool(name="psp1", bufs=3, space="PSUM") as psp1:
                    xqT = xq_pool.tile([P, DC, NQ], f32r)
                    for dc in range(DC):
                        nc.sync.dma_start(out=xqT[:, dc, :], in_=xqT_r[:, dc, :])
                    for hdt in range(DC):
                        wq_t = wst1.tile([P, DC, P], f32r, tag="wstream")
                        nc.sync.dma_start(
                            out=wq_t, in_=wq_r[:, :, hdt * P : (hdt + 1) * P]
                        )
                        for nch in range(NQ // 512):
                            ps = psp1.tile([P, 512], f32, tag="psproj")
                            for dc in range(DC):
                                nc.tensor.matmul(
                                    ps,
                                    wq_t[:, dc, :],
                                    xqT[:, dc, nch * 512 : (nch + 1) * 512],
                                    start=(dc == 0),
                                    stop=(dc == DC - 1),
                                )
                            nc.vector.tensor_copy(
                                qT[:, hdt, nch * 512 : (nch + 1) * 512], ps
                            )

                qproj_scope.__exit__(None, None, None)
                kproj_scope = nc.named_scope("kproj"); kproj_scope.__enter__()
                # ---------------- phase 3: K projection ----------------
                with tc.tile_pool(name="wst3", bufs=2) as wst3, \
                     tc.tile_pool(name="psp3", bufs=3, space="PSUM") as psp3:
                    nch_sizes = []
                    off = 0
                    while off < nk:
                        sz = min(512, nk - off)
                        if nk - (off + sz) == 128:  # avoid a 128-wide tail
                            sz = 384
                        nch_sizes.append((off, sz))
                        off += sz
                    for hdt in range(DC):
                        wk_t = wst3.tile([P, DC, P], f32r, tag="wstream3")
                        nc.sync.dma_start(
                            out=wk_t, in_=wk_r[:, :, hdt * P : (hdt + 1) * P]
                        )
                        for off, sz in nch_sizes:
                            ps = psp3.tile([P, 512], f32, tag="psproj3")
                            for dc in range(DC):
                                nc.tensor.matmul(
                                    ps[:, 0:sz],
                                    wk_t[:, dc, :],
                                    xkT[:, dc, off : off + sz],
                                    start=(dc == 0),
                                    stop=(dc == DC - 1),
                                )
                            nc.vector.tensor_copy(
                                kT[:, hdt, off : off + sz], ps[:, 0:sz]
                            )

            kproj_scope.__exit__(None, None, None)
            attn_scope = nc.named_scope("attn"); attn_scope.__enter__()
            # ---------------- phase 4a: attention heads ----------------
            attn_pool_cm = tc.tile_pool(name="attn_pool", bufs=1)
            attn_pool = attn_pool_cm.__enter__()
            attnT = attn_pool.tile([P, DC, NQ], f32r)
            with tc.tile_pool(name="psS", bufs=4, space="PSUM") as psS_pool, \
                 tc.tile_pool(name="psO", bufs=2, space="PSUM") as psO_pool, \
                 tc.tile_pool(name="pts", bufs=3) as pts, \
                 tc.tile_pool(name="nrm", bufs=1) as nrm:
                for h in range(H):
                    hdt, hh = h // 2, h % 2
                    pbase = DH * hh
                    psO = psO_pool.tile([DH + 2, NQ], f32, tag="psO",
                                        name=f"psOh{h % 2}")
                    prev = None
                    for jt in range(njt_act + 1):
                        cur = []
                        if jt < njt_act:
                            for q2 in range(NQ // 512):
                                qsl = slice(q2 * 512, (q2 + 1) * 512)
                                psS = psS_pool.tile([P, 512], f32, tag="psS",
                                                    name=f"psS{q2}")
                                nc.tensor.matmul(
                                    psS,
                                    kT[pbase : pbase + DH, hdt,
                                       jt * P : (jt + 1) * P],
                                    qT[pbase : pbase + DH, hdt, qsl],
                                    start=True,
                                    stop=True,
                                )
                                cur.append((q2, qsl, psS))
                        if prev is not None:
                            pjt, plist = prev
                            for q2, qsl, pT in plist:
                                nc.tensor.matmul(
                                    psO[:, qsl],
                                    vaug[:, pjt, h, :],
                                    pT,
                                    start=(pjt == 0),
                                    stop=(pjt == njt_act - 1),
                                )
                        if jt < njt_act:
                            plist = []
                            for q2, qsl, psS in cur:
                                pTf = pts.tile([P, 512], f32, tag=f"pTf{q2}")
                                nc.scalar.activation(
                                    pTf,
                                    psS,
                                    mybir.ActivationFunctionType.Exp,
                                    bias=cmneg[:, jt : jt + 1],
                                    scale=SCALE,
                                )
                                pT = pts.tile([P, 512], f32r, tag=f"pT{q2}")
                                nc.vector.tensor_copy(pT, pTf.bitcast(f32r))
                                plist.append((q2, qsl, pT))
                            prev = (jt, plist)
                    # 1/s = exp(-ln(s)) on ACT (both tables in one set)
                    lns = nrm.tile([1, NQ], f32, tag="lns")
                    nc.scalar.activation(
                        lns, psO[DH : DH + 1, :],
                        mybir.ActivationFunctionType.Ln,
                    )
                    r_row = nrm.tile([1, NQ], f32r, tag=f"r_row{h % 2}")
                    nc.scalar.activation(
                        r_row, lns,
                        mybir.ActivationFunctionType.Exp, scale=-1.0,
                    )
                    # broadcast 1/s on idle GpSimd, then normalize while
                    # copying out of PSUM (inputs share start partition 0)
                    rb_sb = nrm.tile([DH, NQ], f32r, tag=f"rb_sb{h % 2}")
                    nc.gpsimd.partition_broadcast(rb_sb, r_row, channels=DH)
                    nc.vector.tensor_mul(
                        attnT[pbase : pbase + DH, hdt, :],
                        psO[0:DH, :],
                        rb_sb,
                    )
            attn_scope.__exit__(None, None, None)
            fin_scope = nc.named_scope("final"); fin_scope.__enter__()
            # -------- phase 5: output projection + masked-query fill --------
            with tc.tile_pool(name="wo_pool", bufs=1) as wo_pool, \
                 tc.tile_pool(name="fin", bufs=3) as fin, \
                 tc.tile_pool(name="psF", bufs=2, space="PSUM") as psF_pool, \
                 tc.tile_pool(name="psU", bufs=1, space="PSUM") as psU_pool:
                wo_sb = wo_pool.tile([P, DC, D], f32r)
                for dc in range(DC):
                    nc.sync.dma_start(out=wo_sb[:, dc, :], in_=wo_r[:, dc, :])
                # uniform_row = meanV @ Wo  [1, 768]
                ps_u1 = psU_pool.tile([1, D], f32, tag="ps_u1")
                for c in range(DC):
                    nc.tensor.matmul(
                        ps_u1[:, 0:512],
                        mvT_sb[:, c : c + 1],
                        wo_sb[:, c, 0:512],
                        start=(c == 0),
                        stop=(c == DC - 1),
                    )
                for c in range(DC):
                    nc.tensor.matmul(
                        ps_u1[:, 512:768],
                        mvT_sb[:, c : c + 1],
                        wo_sb[:, c, 512:768],
                        start=(c == 0),
                        stop=(c == DC - 1),
                    )
                urow_sb = fin.tile([1, D], f32r, tag="urow")
                nc.vector.tensor_copy(urow_sb, ps_u1)

                for it in range(NIT):
                    psF = psF_pool.tile([P, D], f32, tag="psF")
                    for c in range(DC):
                        nc.tensor.matmul(
                            psF[:, 0:512],
                            attnT[:, c, it * P : (it + 1) * P],
                            wo_sb[:, c, 0:512],
                            start=(c == 0),
                            stop=(c == DC - 1),
                        )
                    for c in range(DC):
                        nc.tensor.matmul(
                            psF[:, 512:768],
                            attnT[:, c, it * P : (it + 1) * P],
                            wo_sb[:, c, 512:768],
                            start=(c == 0),
                            stop=(c == DC - 1),
                        )
                    # uniform filler for masked queries: (1-rm01) (x) urow
                    psu = psU_pool.tile([P, D], f32, tag="psu")
                    nc.tensor.matmul(
                        psu[:, 0:512],
                        rmneg_row[0:1, it * P : (it + 1) * P],
                        urow_sb[0:1, 0:512],
                        start=True,
                        stop=True,
                    )
                    nc.tensor.matmul(
                        psu[:, 512:768],
                        rmneg_row[0:1, it * P : (it + 1) * P],
                        urow_sb[0:1, 512:768],
                        start=True,
                        stop=True,
                    )
                    sel_sb = fin.tile([P, D], f32, tag="sel")
                    nc.vector.tensor_scalar_mul(
                        sel_sb, in0=psF, scalar1=rm01[:, it : it + 1]
                    )
                    out_sb = fin.tile([P, D], f32, tag="outsb")
                    nc.vector.tensor_add(out_sb, sel_sb, psu)
                    nc.sync.dma_start(
                        out=out_d.ap()[it * P : (it + 1) * P, :], in_=out_sb
                    )
            fin_scope.__exit__(None, None, None)
            attn_pool_cm.__exit__(None, None, None)

    nc.compile()
    _BUILD_CACHE[njt_act] = nc
    return nc


def _marshal(x, x_mask, Wq, Wk, Wv, Wo):
    """Build per-core input maps. Returns (in_maps, njt_act)."""
    x = np.asarray(x, dtype=np.float32)
    x_mask = np.asarray(x_mask).astype(bool)
    Wq = np.ascontiguousarray(np.asarray(Wq, dtype=np.float32))
    Wk = np.ascontiguousarray(np.asarray(Wk, dtype=np.float32))
    Wv = np.ascontiguousarray(np.asarray(Wv, dtype=np.float32))
    Wo = np.ascontiguousarray(np.asarray(Wo, dtype=np.float32))

    if SORT_KEYS:
        # per-batch stable sort: unmasked keys first
        orders = [np.argsort(~x_mask[b], kind="stable") for b in range(B)]
        counts = [int(x_mask[b].sum()) for b in range(B)]
        njt_act = max(1, -(-max(counts) // P))  # ceil(max unmasked / 128)
    else:
        orders = [np.arange(N) for _ in range(B)]
        njt_act = NJT_FULL

    in_maps = []
    for c in range(8):
        b, qh = c // 2, c % 2
        order = orders[b]
        xk = x[b][order]                       # [N, D] keys (sorted)
        mk = x_mask[b][order]                  # [N] key mask (sorted)
        xq = x[b, qh * NQ : (qh + 1) * NQ]     # [NQ, D] queries natural
        mq = x_mask[b, qh * NQ : (qh + 1) * NQ]

        cm = np.where(mk, 0.0, MASK_NEG).astype(np.float32)      # [N]
        cmnegT = np.ascontiguousarray(cm.reshape(NJT_FULL, P).T)  # [128, 16]
        rm = mq.astype(np.float32)                                # [NQ]
        rm01T = np.ascontiguousarray(rm.reshape(NIT, P).T)        # [128, 8]
        rmneg_row = np.ascontiguousarray((1.0 - rm).reshape(1, NQ))

        in_maps.append({
            "xkT": np.ascontiguousarray(xk.T),   # [768, 2048]
            "xqT": np.ascontiguousarray(xq.T),   # [768, 1024]
            "Wq": Wq, "Wk": Wk, "Wv": Wv, "Wo": Wo,
            "cmnegT": cmnegT,
            "rm01T": rm01T,
            "rmneg_row": rmneg_row,
        })
    return in_maps, njt_act


def run(x, x_mask, Wq, Wk, Wv, Wo, trace=False, tmpdir=None):
    """Run on 8 cores; returns (full_output, BassKernelResults)."""
    in_maps, njt_act = _marshal(x, x_mask, Wq, Wk, Wv, Wo)
    nc = build(njt_act)
    res = run_bass_kernel_spmd(
        nc, in_maps, core_ids=list(range(8)), trace=trace, tmpdir=tmpdir
    )
    out = np.empty((B, N, D), dtype=np.float32)
    for c in range(8):
        b, qh = c // 2, c % 2
        out[b, qh * NQ : (qh + 1) * NQ] = res.results[c]["out"]
    return out, res


def kernel(**inputs) -> np.ndarray:
    out, _ = run(
        inputs["x"], inputs["x_mask"],
        inputs["Wq"], inputs["Wk"], inputs["Wv"], inputs["Wo"],
        trace=False,
    )
    return out



# revision 5
# speedup vs baseline: 2.6693x; 2.6693x over previous
"""Trainium2 Bass kernel for nn_Attention_73375221285454.

Multi-head self-attention (B=4, N=2048, D=768, H=12, DH=64) with key-padding
mask, distributed over 8 NeuronCores.

Sharding: core c handles batch b = c//2 and HEAD GROUP g = c%2 (6 of the 12
heads; each core owns the matching 384 columns of Wq/Wk/Wv and 384 rows of
Wo).  The two cores of a batch produce partial outputs that the host SUMS
(cheap elementwise add) -- no device collective, and no duplicated K/V
projection work.

Host marshalling per batch: rows are sorted so unmasked rows come first
(queries and keys share x and the mask, so ONE sorted xT array serves both),
truncated/padded to njt*128 rows.  Attention is permutation-invariant over
keys, and sorted-out masked queries are reconstructed on the host: a masked
query row equals uniform-softmax over ALL keys, i.e. (mean(x) @ Wv) @ Wo --
a rank-1 O(D^2) row computed on the host and broadcast into masked rows.
Everything is converted to fp16 on the host (halves DMA; matmul throughput
equals f32r at >=256 moving cols; ~tf32 precision; exp(logit) <= ~e^10 fits
fp16 range easily).

Device algorithm per core (all matmuls fp16 operands, f32 PSUM):
  K^T aug = [Wk6.T @ xT ; cmneg]   kT65 [65, 6, NK] (row 64 = -30000*masked)
  Q^T aug = [0.125 * Wq6.T @ xT ; ones]  qT65 (scale folded into q)
  V aug   = [xT.T @ Wv6 ; ones]    vaug [128, njt, 6, 65] (col 64 = ones)
  per (query-chunk qc<=512, head h):
    for groups of key tiles (group size G = 1024//qc):
      S^T = kT65_h.T @ qT65_h      -> psS [128, G*qc]  (mask via aug row!)
      P   = exp(S^T)               -> ONE ACT instr per group, fp16 out
      psO[0:65] += vaug.T @ P      (row 64 accumulates the softmax denom s)
    r = 1/s on DVE (reciprocal), broadcast on GpSimd, normalize on DVE
    attnT packed [128, 3, NK]; odd heads repartitioned 0:64->64:128 via a
    tiny SBUF->SBUF DMA (DVE cannot shift partitions).
  out = attnT.T @ Wo6  -> [NK, 768] fp16 -> host scatter+add.

The aug-row trick removes the per-key-tile bias AP from exp, so exp batches
across key tiles: ~3x fewer ACT instructions (ACT is the critical engine:
heads*nk*nq exp elements at ~0.83 ns/col + ~240 ns/instr overhead).
"""

import sys

sys.path.insert(0, "/opt/trn_rl_repo")

import numpy as np

import concourse.bass as bass  # noqa: F401
import concourse.mybir as mybir
import concourse.tile as tile
from concourse import bacc
from concourse.bass_utils import run_bass_kernel_spmd

P = 128
B, N, D = 4, 2048, 768
H, DH = 12, 64
H6 = 6                   # heads per core
HD6 = H6 * DH            # 384 projected dims per core
DC = D // P              # 6 contraction chunks of the model dim
MASK_NEG = -30000.0

f16 = mybir.dt.float16
f32 = mybir.dt.float32

_BUILD_CACHE = {}


def build(njt: int) -> "bacc.Bacc":
    """Build the SPMD program for njt active 128-row tiles (keys==queries)."""
    if njt in _BUILD_CACHE:
        return _BUILD_CACHE[njt]

    NK = njt * P

    nc = bacc.Bacc()
    xT_d = nc.declare_dram_parameter("xT", [D, NK], f16, isOutput=False)
    wq_d = nc.declare_dram_parameter("Wq6", [D, HD6], f16, isOutput=False)
    wk_d = nc.declare_dram_parameter("Wk6", [D, HD6], f16, isOutput=False)
    wv_d = nc.declare_dram_parameter("Wv6", [D, HD6], f16, isOutput=False)
    wo_d = nc.declare_dram_parameter("Wo6", [HD6, D], f16, isOutput=False)
    # cmneg[0, j] = 0.0 if row j unmasked else -30000.0 (sorted order)
    cm_d = nc.declare_dram_parameter("cmneg", [1, NK], f16, isOutput=False)
    out_d = nc.declare_dram_parameter("out", [NK, D], f16, isOutput=True)

    xT_r = xT_d.rearrange("(c p) n -> p c n", p=P)
    wq_r = wq_d.rearrange("(c p) e -> p c e", p=P)
    wk_r = wk_d.rearrange("(c p) e -> p c e", p=P)
    wv_r = wv_d.rearrange("(c p) e -> p c e", p=P)
    wo_r = wo_d.rearrange("(c p) e -> p c e", p=P)

    # query chunks (<=512 cols, PSUM-bank aligned)
    qcs = [(off, min(512, NK - off)) for off in range(0, NK, 512)]
    # K/Q projection column chunks over NK
    kchunks = qcs

    with tile.TileContext(nc) as tc:
        with tc.tile_pool(name="persist", bufs=1) as persist:
            kT65 = persist.tile([P, H6, NK], f16)   # rows 0:64 K, row 64 cmneg
            qT65 = persist.tile([P, H6, NK], f16)   # rows 0:64 q/8, row 64 one
            vaug = persist.tile([P, njt, H6, DH + 1], f16)
            attnT = persist.tile([P, 3, NK], f16)
            wo_sb = persist.tile([P, 3, D], f16)

            proj_scope = nc.named_scope("proj"); proj_scope.__enter__()
            with tc.tile_pool(name="projsb", bufs=1) as pp, \
                 tc.tile_pool(name="psv_p", bufs=2, space="PSUM") as psv_p, \
                 tc.tile_pool(name="pskq_p", bufs=2, space="PSUM") as pskq_p:
                xT_sb = pp.tile([P, DC, NK], f16)
                wv_sb = pp.tile([P, DC, HD6], f16)
                wk_sb = pp.tile([P, DC, HD6], f16)
                wq_sb = pp.tile([P, DC, HD6], f16)
                kstage = pp.tile([P, 3, NK], f16)
                qstage = pp.tile([P, 3, NK], f16)

                # ---- input DMAs (wv + x first so V matmuls start early) ----
                for dc in range(DC):
                    nc.sync.dma_start(out=wv_sb[:, dc, :], in_=wv_r[:, dc, :])
                for (cgo, cgl) in qcs:
                    for dc in range(DC):
                        nc.sync.dma_start(
                            out=xT_sb[:, dc, cgo:cgo + cgl],
                            in_=xT_r[:, dc, cgo:cgo + cgl],
                        )
                for dc in range(DC):
                    nc.sync.dma_start(out=wk_sb[:, dc, :], in_=wk_r[:, dc, :])
                for dc in range(DC):
                    nc.sync.dma_start(out=wq_sb[:, dc, :], in_=wq_r[:, dc, :])
                for c in range(3):
                    nc.sync.dma_start(out=wo_sb[:, c, :], in_=wo_r[:, c, :])
                for h in range(H6):
                    nc.sync.dma_start(out=kT65[64:65, h, :], in_=cm_d.ap())

                # ---------------- V projection -> vaug ----------------
                for jt in range(njt):
                    psv = psv_p.tile([P, HD6], f32, tag="psv")
                    for dc in range(DC):
                        nc.tensor.matmul(
                            psv,
                            xT_sb[:, dc, jt * P:(jt + 1) * P],
                            wv_sb[:, dc, :],
                            start=(dc == 0),
                            stop=(dc == DC - 1),
                        )
                    nc.vector.tensor_copy(
                        vaug[:, jt, :, 0:DH],
                        psv.rearrange("p (h d) -> p h d", h=H6),
                    )
                nc.vector.memset(vaug[:, :, :, DH:DH + 1], 1.0)

                # ---------------- K projection -> kT65 ----------------
                for hdt in range(3):
                    psks = [
                        pskq_p.tile([P, cs], f32, tag=f"psk{i}",
                                    name=f"psk{i}")
                        for i, (off, cs) in enumerate(kchunks)
                    ]
                    for dc in range(DC):
                        for i, (off, cs) in enumerate(kchunks):
                            nc.tensor.matmul(
                                psks[i],
                                wk_sb[:, dc, hdt * P:(hdt + 1) * P],
                                xT_sb[:, dc, off:off + cs],
                                start=(dc == 0),
                                stop=(dc == DC - 1),
                            )
                    for i, (off, cs) in enumerate(kchunks):
                        nc.vector.tensor_copy(
                            kstage[:, hdt, off:off + cs], psks[i]
                        )
                    nc.sync.dma_start(
                        out=kT65[0:DH, 2 * hdt, :], in_=kstage[0:DH, hdt, :]
                    )
                    nc.sync.dma_start(
                        out=kT65[0:DH, 2 * hdt + 1, :],
                        in_=kstage[DH:P, hdt, :],
                    )

                # ------------- Q projection (scaled) -> qT65 -------------
                for hdt in range(3):
                    psqs = [
                        pskq_p.tile([P, cs], f32, tag=f"psk{i}",
                                    name=f"psq{i}")
                        for i, (off, cs) in enumerate(kchunks)
                    ]
                    for dc in range(DC):
                        for i, (off, cs) in enumerate(kchunks):
                            nc.tensor.matmul(
                                psqs[i],
                                wq_sb[:, dc, hdt * P:(hdt + 1) * P],
                                xT_sb[:, dc, off:off + cs],
                                start=(dc == 0),
                                stop=(dc == DC - 1),
                            )
                    for i, (off, cs) in enumerate(kchunks):
                        nc.vector.tensor_scalar_mul(
                            qstage[:, hdt, off:off + cs],
                            in0=psqs[i],
                            scalar1=DH ** -0.5,
                        )
                    nc.sync.dma_start(
                        out=qT65[0:DH, 2 * hdt, :], in_=qstage[0:DH, hdt, :]
                    )
                    nc.sync.dma_start(
                        out=qT65[0:DH, 2 * hdt + 1, :],
                        in_=qstage[DH:P, hdt, :],
                    )
                for h in range(H6):
                    nc.vector.memset(qT65[64:65, h, :], 1.0)
            proj_scope.__exit__(None, None, None)

            # ---------------- attention ----------------
            attn_scope = nc.named_scope("attn"); attn_scope.__enter__()
            with tc.tile_pool(name="psS_p", bufs=2, space="PSUM") as psS_p, \
                 tc.tile_pool(name="psO_p", bufs=2, space="PSUM") as psO_p, \
                 tc.tile_pool(name="pts", bufs=3) as pts, \
                 tc.tile_pool(name="nrm", bufs=2) as nrm:
                for (qoff, qlen) in qcs:
                    G = max(1, 1024 // qlen)
                    groups = [
                        list(range(j, min(j + G, njt)))
                        for j in range(0, njt, G)
                    ]
                    for h in range(H6):
                        psO = psO_p.tile([DH + 1, qlen], f32, tag="psO")
                        prev = None
                        for grp in groups + [None]:
                            if grp is not None:
                                glen = len(grp) * qlen
                                psS = psS_p.tile([P, glen], f32, tag="psS")
                                for gi, jt in enumerate(grp):
                                    nc.tensor.matmul(
                                        psS[:, gi * qlen:(gi + 1) * qlen],
                                        kT65[0:DH + 1, h,
                                             jt * P:(jt + 1) * P],
                                        qT65[0:DH + 1, h, qoff:qoff + qlen],
                                        start=True,
                                        stop=True,
                                    )
                                pT = pts.tile([P, glen], f16, tag="pT")
                                nc.scalar.activation(
                                    pT, psS,
                                    mybir.ActivationFunctionType.Exp,
                                )
                            if prev is not None:
                                pgrp, ppT = prev
                                for gi, jt in enumerate(pgrp):
                                    nc.tensor.matmul(
                                        psO,
                                        vaug[:, jt, h, :],
                                        ppT[:, gi * qlen:(gi + 1) * qlen],
                                        start=(jt == 0),
                                        stop=(jt == njt - 1),
                                    )
                            if grp is not None:
                                prev = (grp, pT)
                        # normalize: r = 1/s on DVE, broadcast on Pool
                        r_row = nrm.tile([1, qlen], f32, tag="r")
                        nc.vector.reciprocal(r_row, psO[DH:DH + 1, :])
                        rb = nrm.tile([DH, qlen], f32, tag="rb")
                        nc.gpsimd.partition_broadcast(rb, r_row, channels=DH)
                        hdt = h // 2
                        if h % 2 == 0:
                            nc.vector.tensor_mul(
                                attnT[0:DH, hdt, qoff:qoff + qlen],
                                psO[0:DH, :],
                                rb,
                            )
                        else:
                            ast = nrm.tile([DH, qlen], f16, tag="ast")
                            nc.vector.tensor_mul(ast, psO[0:DH, :], rb)
                            nc.sync.dma_start(
                                out=attnT[DH:P, hdt, qoff:qoff + qlen],
                                in_=ast,
                            )
            attn_scope.__exit__(None, None, None)

            # ---------------- output projection ----------------
            fin_scope = nc.named_scope("final"); fin_scope.__enter__()
            with tc.tile_pool(name="psF_p", bufs=2, space="PSUM") as psF_p, \
                 tc.tile_pool(name="fin", bufs=2) as fin:
                for it in range(njt):
                    psF = psF_p.tile([P, D], f32, tag="psF")
                    for c in range(3):
                        nc.tensor.matmul(
                            psF[:, 0:512],
                            attnT[:, c, it * P:(it + 1) * P],
                            wo_sb[:, c, 0:512],
                            start=(c == 0),
                            stop=(c == 2),
                        )
                    for c in range(3):
                        nc.tensor.matmul(
                            psF[:, 512:768],
                            attnT[:, c, it * P:(it + 1) * P],
                            wo_sb[:, c, 512:768],
                            start=(c == 0),
                            stop=(c == 2),
                        )
                    outsb = fin.tile([P, D], f16, tag="outsb")
                    nc.vector.tensor_copy(outsb, psF)
                    nc.sync.dma_start(
                        out=out_d.ap()[it * P:(it + 1) * P, :], in_=outsb
                    )
            fin_scope.__exit__(None, None, None)

    nc.compile()
    _BUILD_CACHE[njt] = nc
    return nc


def _marshal(x, x_mask, Wq, Wk, Wv, Wo):
    """Build per-core input maps. Returns (in_maps, njt, orders, counts,
    urows)."""
    x = np.asarray(x, dtype=np.float32)
    x_mask = np.asarray(x_mask).astype(bool)
    Wq = np.asarray(Wq, dtype=np.float32)
    Wk = np.asarray(Wk, dtype=np.float32)
    Wv = np.asarray(Wv, dtype=np.float32)
    Wo = np.asarray(Wo, dtype=np.float32)

    orders = [np.argsort(~x_mask[b], kind="stable") for b in range(B)]
    counts = [int(x_mask[b].sum()) for b in range(B)]
    njt = max(1, -(-max(counts) // P))
    NK = njt * P

    # uniform row for fully-masked queries: (mean_all_rows x @ Wv) @ Wo
    urows = (x.mean(axis=1) @ Wv) @ Wo  # [B, D] f32

    w16 = {
        "Wq": Wq.astype(np.float16),
        "Wk": Wk.astype(np.float16),
        "Wv": Wv.astype(np.float16),
        "Wo": Wo.astype(np.float16),
    }

    in_maps = []
    for c in range(8):
        b, g = c // 2, c % 2
        sel = orders[b][:NK]
        xT16 = np.ascontiguousarray(x[b][sel].T.astype(np.float16))
        cm16 = np.where(x_mask[b][sel], 0.0, MASK_NEG).astype(
            np.float16
        ).reshape(1, NK)
        cols = slice(g * HD6, (g + 1) * HD6)
        in_maps.append({
            "xT": xT16,
            "Wq6": np.ascontiguousarray(w16["Wq"][:, cols]),
            "Wk6": np.ascontiguousarray(w16["Wk"][:, cols]),
            "Wv6": np.ascontiguousarray(w16["Wv"][:, cols]),
            "Wo6": np.ascontiguousarray(w16["Wo"][cols, :]),
            "cmneg": np.ascontiguousarray(cm16),
        })
    return in_maps, njt, orders, counts, urows


def run(x, x_mask, Wq, Wk, Wv, Wo, trace=False, tmpdir=None):
    """Run on 8 cores; returns (full_output, BassKernelResults)."""
    in_maps, njt, orders, counts, urows = _marshal(x, x_mask, Wq, Wk, Wv, Wo)
    nc = build(njt)
    res = run_bass_kernel_spmd(
        nc, in_maps, core_ids=list(range(8)), trace=trace, tmpdir=tmpdir
    )
    out = np.empty((B, N, D), dtype=np.float32)
    for b in range(B):
        out[b, :, :] = urows[b]
        cnt = counts[b]
        if cnt:
            act = orders[b][:cnt]
            out[b, act] = (
                res.results[2 * b]["out"][:cnt].astype(np.float32)
                + res.results[2 * b + 1]["out"][:cnt].astype(np.float32)
            )
    return out, res


def kernel(**inputs) -> np.ndarray:
    out, _ = run(
        inputs["x"], inputs["x_mask"],
        inputs["Wq"], inputs["Wk"], inputs["Wv"], inputs["Wo"],
        trace=False,
    )
    return out


# revision 11
# speedup vs baseline: 2.9012x; 1.0869x over previous
"""Trainium2 Bass kernel for nn_Attention_73375221285454.

Multi-head self-attention (B=4, N=2048, D=768, H=12, DH=64) with key-padding
mask, distributed over 8 NeuronCores.

Sharding: core c handles batch b = c//2 and HEAD GROUP g = c%2 (6 of the 12
heads; each core owns the matching 384 columns of Wq/Wk/Wv and 384 rows of
Wo).  The two cores of a batch produce partial outputs that the host SUMS
(cheap elementwise add) -- no device collective, and no duplicated K/V
projection work.

Host marshalling per batch: rows are sorted so unmasked rows come first
(queries and keys share x and the mask, so ONE sorted xT array serves both),
padded to njt*128 rows.  Attention is permutation-invariant over keys, and
masked queries are reconstructed on the host: a masked query row equals
uniform-softmax over ALL keys, i.e. (mean(x) @ Wv) @ Wo -- a rank-1 O(D^2)
row computed on the host and broadcast into masked rows.  Everything is fp16
on the host (halves DMA; PE throughput equals f32r at >=256 moving cols;
~tf32 precision; exp(logit) fits fp16 range easily).

Device algorithm per core (fp16 operands, f32 PSUM):
  K^T aug = [Wk6.T @ xT ; cmneg]  kT65 [65, 6, NK] (row 64 = -30000*masked)
  Q^T aug = [0.125*Wq6.T @ xT ; 1] qT65 (softmax scale folded into q)
  V aug   = [xT.T @ Wv6 ; ones]   vaug [128, njt, 6, 65] (col 64 => denom s)
  MAIN (queries 0:1024): per (head h, key tile jt):
     one LDW + S^T = kT65_h.T @ qT65_h (2x512) -> psS [128,1024]
     ONE exp instr  P = exp(S^T) -> fp16 (mask via the aug row: no bias AP)
     one LDW + psO[0:65] += vaug_jt.T @ P (2x512; row 64 accumulates s)
  TAIL (queries 1024:1024+nq_tail, nq_tail = max_count-1024, e.g. 20):
     same but grouped: all key tiles in <=2 psS tiles, 1-2 exp instrs.
  normalize: 1/s via DVE reciprocal_approx_fast + GpSimd broadcast + DVE
  mul; odd heads repartitioned 0:64->64:128 via tiny SBUF->SBUF DMA.
  out = attnT.T @ Wo6 -> [<=1024+nq_tail, 768] fp16 -> host scatter+add.

Engine budget notes: ACT does ONLY exp (54 x [128,1024] + tail) ~62us; PE
does ~86us of matmul columns + ~25us of LDWEIGHTS (stationaries shared
across the 2x512 moving chunks); DVE/GpSimd/DMA hide under those.  K/Q
projection chunks are interleaved between attention heads so exp starts
~20us earlier; DMA issues are consolidated (few, large) and split between
the Sync queue (inputs/repartition/out) and ACT queue (attnT odd-half) to
avoid head-of-line blocking.
"""

import sys

sys.path.insert(0, "/opt/trn_rl_repo")

import numpy as np

import concourse.bass as bass  # noqa: F401
import concourse.mybir as mybir
import concourse.tile as tile
from concourse import bacc
from concourse.bass_utils import run_bass_kernel_spmd

P = 128
B, N, D = 4, 2048, 768
H, DH = 12, 64
H6 = 6                   # heads per core
HD6 = H6 * DH            # 384 projected dims per core
DC = D // P              # 6 contraction chunks of the model dim
MASK_NEG = -30000.0

f16 = mybir.dt.float16
f32 = mybir.dt.float32

_BUILD_CACHE = {}


def build(njt: int, nq_tail: int) -> "bacc.Bacc":
    """Build the SPMD program.

    njt: number of 128-row key/query tiles (sorted, active first).
    nq_tail: query columns beyond 1024 that contain real (unmasked) queries.
    """
    key = (njt, nq_tail)
    if key in _BUILD_CACHE:
        return _BUILD_CACHE[key]

    NK = njt * P
    QM = min(NK, 1024)           # main-phase query columns
    NQ = QM + nq_tail            # total query columns computed

    nc = bacc.Bacc()
    xT_d = nc.declare_dram_parameter("xT", [D, NK], f16, isOutput=False)
    wq_d = nc.declare_dram_parameter("Wq6", [D, HD6], f16, isOutput=False)
    wk_d = nc.declare_dram_parameter("Wk6", [D, HD6], f16, isOutput=False)
    wv_d = nc.declare_dram_parameter("Wv6", [D, HD6], f16, isOutput=False)
    wo_d = nc.declare_dram_parameter("Wo6", [HD6, D], f16, isOutput=False)
    cm_d = nc.declare_dram_parameter("cmneg", [1, NK], f16, isOutput=False)
    out_d = nc.declare_dram_parameter("out", [NQ, D], f16, isOutput=True)

    xT_r = xT_d.rearrange("(c p) n -> p c n", p=P)
    wq_r = wq_d.rearrange("(c p) e -> p c e", p=P)
    wk_r = wk_d.rearrange("(c p) e -> p c e", p=P)
    wv_r = wv_d.rearrange("(c p) e -> p c e", p=P)
    wo_r = wo_d.rearrange("(c p) e -> p c e", p=P)

    # K/Q projection column chunks over NK (PSUM <=512)
    kchunks = [(off, min(512, NK - off)) for off in range(0, NK, 512)]

    with tile.TileContext(nc) as tc:
        with tc.tile_pool(name="persist", bufs=1) as persist:
            kT65 = persist.tile([P, H6, NK], f16)   # rows 0:64 K, row 64 cmneg
            qT65 = persist.tile([P, H6, NK], f16)   # rows 0:64 q/8, row 64 one
            vaug = persist.tile([P, njt, H6, DH + 1], f16)
            attnT = persist.tile([P, 3, NQ], f16)
            wo_sb = persist.tile([P, 3, D], f16)

            pp_cm = tc.tile_pool(name="projsb", bufs=1)
            pp = pp_cm.__enter__()
            pskq_cm = tc.tile_pool(name="pskq_p", bufs=2, space="PSUM")
            pskq_p = pskq_cm.__enter__()

            xT_sb = pp.tile([P, DC, NK], f16)
            wv_sb = pp.tile([P, DC, HD6], f16)
            wk_sb = pp.tile([P, DC, HD6], f16)
            wq_sb = pp.tile([P, DC, HD6], f16)
            kstage = pp.tile([P, 3, NK], f16)
            qstage = pp.tile([P, 3, NK], f16)

            # ---- consolidated input DMAs (few, large; Sync queue) ----
            nc.sync.dma_start(out=wv_sb, in_=wv_r)
            xcg = 384
            for cgo in range(0, NK, xcg):
                cgl = min(xcg, NK - cgo)
                nc.sync.dma_start(
                    out=xT_sb[:, :, cgo:cgo + cgl],
                    in_=xT_r[:, :, cgo:cgo + cgl],
                )
            nc.sync.dma_start(out=wk_sb, in_=wk_r)
            nc.sync.dma_start(out=wq_sb, in_=wq_r)
            nc.sync.dma_start(out=wo_sb, in_=wo_r)
            for h in range(H6):
                nc.sync.dma_start(out=kT65[64:65, h, :], in_=cm_d.ap())

            # ones rows/cols early (no deps)
            nc.vector.memset(vaug[:, :, :, DH:DH + 1], 1.0)
            for h in range(H6):
                nc.vector.memset(qT65[64:65, h, :], 1.0)

            def proj_chunk(hdt, w_sb, stage, dest, scale):
                """Project one 128-col chunk of Wk/Wq; stage + repartition."""
                psks = [
                    pskq_p.tile([P, cs], f32, tag=f"pskq{i}",
                                name=f"pskq{hdt}_{i}")
                    for i, (off, cs) in enumerate(kchunks)
                ]
                for dc in range(DC):
                    for i, (off, cs) in enumerate(kchunks):
                        nc.tensor.matmul(
                            psks[i],
                            w_sb[:, dc, hdt * P:(hdt + 1) * P],
                            xT_sb[:, dc, off:off + cs],
                            start=(dc == 0),
                            stop=(dc == DC - 1),
                        )
                for i, (off, cs) in enumerate(kchunks):
                    if scale is None:
                        nc.vector.tensor_copy(
                            stage[:, hdt, off:off + cs], psks[i]
                        )
                    else:
                        nc.vector.tensor_scalar_mul(
                            stage[:, hdt, off:off + cs],
                            in0=psks[i],
                            scalar1=scale,
                        )
                nc.sync.dma_start(
                    out=dest[0:DH, 2 * hdt, :], in_=stage[0:DH, hdt, :]
                )
                nc.sync.dma_start(
                    out=dest[0:DH, 2 * hdt + 1, :], in_=stage[DH:P, hdt, :]
                )

            # ---------------- projections (sequential) ----------------
            proj_scope = nc.named_scope("proj"); proj_scope.__enter__()
            with tc.tile_pool(name="psv_p", bufs=2, space="PSUM") as psv_p:
                for jt in range(njt):
                    psv = psv_p.tile([P, HD6], f32, tag="psv", name="psv")
                    for dc in range(DC):
                        nc.tensor.matmul(
                            psv,
                            xT_sb[:, dc, jt * P:(jt + 1) * P],
                            wv_sb[:, dc, :],
                            start=(dc == 0),
                            stop=(dc == DC - 1),
                        )
                    nc.vector.tensor_copy(
                        vaug[:, jt, :, 0:DH],
                        psv.rearrange("p (h d) -> p h d", h=H6),
                    )
                for hdt in range(3):
                    proj_chunk(hdt, wk_sb, kstage, kT65, None)
                    proj_chunk(hdt, wq_sb, qstage, qT65, DH ** -0.5)
            proj_scope.__exit__(None, None, None)

            pskq_cm.__exit__(None, None, None)
            pp_cm.__exit__(None, None, None)

            # attention pools (PSUM: psS 2x2 + psO 2x2 = 8 banks)
            psS_cm = tc.tile_pool(name="psS_p", bufs=2, space="PSUM")
            psS_p = psS_cm.__enter__()
            psO_cm = tc.tile_pool(name="psO_p", bufs=2, space="PSUM")
            psO_p = psO_cm.__enter__()
            pts_cm = tc.tile_pool(name="pts", bufs=3)
            pts = pts_cm.__enter__()
            nrm_cm = tc.tile_pool(name="nrm", bufs=2)
            nrm = nrm_cm.__enter__()

            def attn_head(h, qoff, qlen, glen_max):
                """Attention for head h over queries [qoff, qoff+qlen)."""
                hdt = h // 2
                psO = psO_p.tile([DH + 1, qlen], f32, tag="psO", name="psO")
                G = max(1, glen_max // qlen)
                groups = [
                    list(range(j, min(j + G, njt))) for j in range(0, njt, G)
                ]
                prev = None
                for grp in groups + [None]:
                    if grp is not None:
                        glen = len(grp) * qlen
                        psS = psS_p.tile([P, glen], f32, tag="psS",
                                         name="psS")
                        for gi, jt in enumerate(grp):
                            for so in range(0, qlen, 512):
                                sl = min(512, qlen - so)
                                nc.tensor.matmul(
                                    psS[:, gi * qlen + so:
                                        gi * qlen + so + sl],
                                    kT65[0:DH + 1, h, jt * P:(jt + 1) * P],
                                    qT65[0:DH + 1, h,
                                         qoff + so:qoff + so + sl],
                                    start=True,
                                    stop=True,
                                )
                        pT = pts.tile([P, glen], f16, tag="pT", name="pT")
                        nc.scalar.activation(
                            pT, psS, mybir.ActivationFunctionType.Exp,
                        )
                    if prev is not None:
                        pgrp, ppT = prev
                        for gi, jt in enumerate(pgrp):
                            for so in range(0, qlen, 512):
                                sl = min(512, qlen - so)
                                # start/stop are per PSUM region: each
                                # 512-col half is its own accumulation group
                                nc.tensor.matmul(
                                    psO[:, so:so + sl],
                                    vaug[:, jt, h, :],
                                    ppT[:, gi * qlen + so:
                                        gi * qlen + so + sl],
                                    start=(jt == 0),
                                    stop=(jt == njt - 1),
                                )
                    if grp is not None:
                        prev = (grp, pT)
                # normalize (DVE fast reciprocal + Pool broadcast + DVE mul)
                r_row = nrm.tile([1, qlen], f32, tag="r", name="r_row")
                nc.vector.reciprocal(r_row, psO[DH:DH + 1, :])
                rb = nrm.tile([DH, qlen], f32, tag="rb", name="rb")
                nc.gpsimd.partition_broadcast(rb, r_row, channels=DH)
                if h % 2 == 0:
                    nc.vector.tensor_mul(
                        attnT[0:DH, hdt, qoff:qoff + qlen], psO[0:DH, :], rb
                    )
                else:
                    ast = nrm.tile([DH, qlen], f16, tag="ast", name="ast")
                    nc.vector.tensor_mul(ast, psO[0:DH, :], rb)
                    nc.sync.dma_start(
                        out=attnT[DH:P, hdt, qoff:qoff + qlen], in_=ast
                    )

            attn_scope = nc.named_scope("attn"); attn_scope.__enter__()
            for h in range(H6):
                attn_head(h, 0, QM, 1024)
            # tail queries (real queries beyond 1024, if any)
            if nq_tail > 0:
                for h in range(H6):
                    attn_head(h, QM, nq_tail, 1024)
            attn_scope.__exit__(None, None, None)

            nrm_cm.__exit__(None, None, None)
            pts_cm.__exit__(None, None, None)
            psO_cm.__exit__(None, None, None)
            psS_cm.__exit__(None, None, None)

            # ---------------- output projection ----------------
            fin_scope = nc.named_scope("final"); fin_scope.__enter__()
            with tc.tile_pool(name="psF_p", bufs=2, space="PSUM") as psF_p, \
                 tc.tile_pool(name="fin", bufs=2) as fin:
                ftiles = [(o, min(P, NQ - o)) for o in range(0, NQ, P)]
                for (fo, fl) in ftiles:
                    psF = psF_p.tile([fl, D], f32, tag="psF", name="psF")
                    for half in (slice(0, 512), slice(512, 768)):
                        for c in range(3):
                            nc.tensor.matmul(
                                psF[:, half],
                                attnT[:, c, fo:fo + fl],
                                wo_sb[:, c, half],
                                start=(c == 0),
                                stop=(c == 2),
                            )
                    outsb = fin.tile([fl, D], f16, tag="outsb", name="outsb")
                    nc.vector.tensor_copy(outsb, psF)
                    nc.sync.dma_start(
                        out=out_d.ap()[fo:fo + fl, :], in_=outsb
                    )
            fin_scope.__exit__(None, None, None)

    nc.compile()
    _BUILD_CACHE[key] = nc
    return nc


def _marshal(x, x_mask, Wq, Wk, Wv, Wo):
    """Build per-core input maps."""
    x = np.asarray(x, dtype=np.float32)
    x_mask = np.asarray(x_mask).astype(bool)
    Wq = np.asarray(Wq, dtype=np.float32)
    Wk = np.asarray(Wk, dtype=np.float32)
    Wv = np.asarray(Wv, dtype=np.float32)
    Wo = np.asarray(Wo, dtype=np.float32)

    orders = [np.argsort(~x_mask[b], kind="stable") for b in range(B)]
    counts = [int(x_mask[b].sum()) for b in range(B)]
    njt = max(1, -(-max(counts) // P))
    NK = njt * P
    nq_tail = max(0, max(counts) - min(NK, 1024))

    # uniform row for fully-masked queries: (mean_all_rows x @ Wv) @ Wo
    urows = (x.mean(axis=1) @ Wv) @ Wo  # [B, D] f32

    w16 = {
        "Wq": Wq.astype(np.float16),
        "Wk": Wk.astype(np.float16),
        "Wv": Wv.astype(np.float16),
        "Wo": Wo.astype(np.float16),
    }

    in_maps = []
    for c in range(8):
        b, g = c // 2, c % 2
        sel = orders[b][:NK]
        xT16 = np.ascontiguousarray(x[b][sel].T.astype(np.float16))
        cm16 = np.where(x_mask[b][sel], 0.0, MASK_NEG).astype(
            np.float16
        ).reshape(1, NK)
        cols = slice(g * HD6, (g + 1) * HD6)
        in_maps.append({
            "xT": xT16,
            "Wq6": np.ascontiguousarray(w16["Wq"][:, cols]),
            "Wk6": np.ascontiguousarray(w16["Wk"][:, cols]),
            "Wv6": np.ascontiguousarray(w16["Wv"][:, cols]),
            "Wo6": np.ascontiguousarray(w16["Wo"][cols, :]),
            "cmneg": np.ascontiguousarray(cm16),
        })
    return in_maps, njt, nq_tail, orders, counts, urows


def run(x, x_mask, Wq, Wk, Wv, Wo, trace=False, tmpdir=None):
    """Run on 8 cores; returns (full_output, BassKernelResults)."""
    in_maps, njt, nq_tail, orders, counts, urows = _marshal(
        x, x_mask, Wq, Wk, Wv, Wo
    )
    nc = build(njt, nq_tail)
    res = run_bass_kernel_spmd(
        nc, in_maps, core_ids=list(range(8)), trace=trace, tmpdir=tmpdir
    )
    out = np.empty((B, N, D), dtype=np.float32)
    for b in range(B):
        out[b, :, :] = urows[b]
        cnt = counts[b]
        if cnt:
            act = orders[b][:cnt]
            out[b, act] = (
                res.results[2 * b]["out"][:cnt].astype(np.float32)
                + res.results[2 * b + 1]["out"][:cnt].astype(np.float32)
            )
    return out, res


def kernel(**inputs) -> np.ndarray:
    out, _ = run(
        inputs["x"], inputs["x_mask"],
        inputs["Wq"], inputs["Wk"], inputs["Wv"], inputs["Wo"],
        trace=False,
    )
    return out


# revision 19
# speedup vs baseline: 2.9700x; 1.0237x over previous
"""Trainium2 Bass kernel for nn_Attention_73375221285454.

Multi-head self-attention (B=4, N=2048, D=768, H=12, DH=64) with key-padding
mask, distributed over 8 NeuronCores.

Sharding: core c handles batch b = c//2 and HEAD GROUP g = c%2 (6 of the 12
heads; each core owns the matching 384 columns of Wq/Wk/Wv and 384 rows of
Wo).  The two cores of a batch produce partial outputs that the host SUMS
(cheap elementwise add) -- no device collective, and no duplicated K/V
projection work.

Host marshalling per batch: rows are sorted so unmasked rows come first
(queries and keys share x and the mask, so ONE sorted xT array serves both),
padded to njt*128 rows.  Attention is permutation-invariant over keys, and
masked queries are reconstructed on the host: a masked query row equals
uniform-softmax over ALL keys, i.e. (mean(x) @ Wv) @ Wo -- a rank-1 O(D^2)
row computed on the host and broadcast into masked rows.  Everything is fp16
on the host (halves DMA; PE throughput equals f32r at >=256 moving cols;
~tf32 precision; exp(logit) fits fp16 range easily).

Device algorithm per core (fp16 operands, f32 PSUM):
  K^T aug = [Wk6.T @ xT ; cmneg]  kT65 [65, 6, NK] (row 64 = -30000*masked)
  Q^T aug = [0.125*Wq6.T @ xT ; 1] qT65 (softmax scale folded into q)
  V aug   = [xT.T @ Wv6 ; ones]   vaug [128, njt, 6, 65] (col 64 => denom s)
  MAIN (queries 0:1024): per (head h, key tile jt):
     one LDW + S^T = kT65_h.T @ qT65_h (2x512) -> psS [128,1024]
     ONE exp instr  P = exp(S^T) -> fp16 (mask via the aug row: no bias AP)
     one LDW + psO[0:65] += vaug_jt.T @ P (2x512; row 64 accumulates s)
  TAIL (queries 1024:1024+nq_tail, nq_tail = max_count-1024, e.g. 20):
     same but grouped: all key tiles in <=2 psS tiles, 1-2 exp instrs.
  normalize: 1/s via DVE reciprocal_approx_fast + GpSimd broadcast + DVE
  mul; odd heads repartitioned 0:64->64:128 via tiny SBUF->SBUF DMA.
  out = attnT.T @ Wo6 -> [<=1024+nq_tail, 768] fp16 -> host scatter+add.

Engine budget notes: ACT does ONLY exp (54 x [128,1024] + tail) ~62us; PE
does ~86us of matmul columns + ~25us of LDWEIGHTS (stationaries shared
across the 2x512 moving chunks); DVE/GpSimd/DMA hide under those.  K/Q
projection chunks are interleaved between attention heads so exp starts
~20us earlier; DMA issues are consolidated (few, large) and split between
the Sync queue (inputs/repartition/out) and ACT queue (attnT odd-half) to
avoid head-of-line blocking.
"""

import sys

sys.path.insert(0, "/opt/trn_rl_repo")

import numpy as np

import concourse.bass as bass  # noqa: F401
import concourse.mybir as mybir
import concourse.tile as tile
from concourse import bacc
from concourse.bass_utils import run_bass_kernel_spmd

P = 128
B, N, D = 4, 2048, 768
H, DH = 12, 64
H6 = 6                   # heads per core
HD6 = H6 * DH            # 384 projected dims per core
DC = D // P              # 6 contraction chunks of the model dim
MASK_NEG = -30000.0

f16 = mybir.dt.float16
f32 = mybir.dt.float32

_BUILD_CACHE = {}


def build(njt: int, nq_tail: int) -> "bacc.Bacc":
    """Build the SPMD program.

    njt: number of 128-row key/query tiles (sorted, active first).
    nq_tail: query columns beyond 1024 that contain real (unmasked) queries.
    """
    key = (njt, nq_tail)
    if key in _BUILD_CACHE:
        return _BUILD_CACHE[key]

    NK = njt * P
    QM = min(NK, 1024)           # main-phase query columns
    NQ = QM + nq_tail            # total query columns computed

    nc = bacc.Bacc()
    xT_d = nc.declare_dram_parameter("xT", [D, NK], f16, isOutput=False)
    wq_d = nc.declare_dram_parameter("Wq6", [D, HD6], f16, isOutput=False)
    wk_d = nc.declare_dram_parameter("Wk6", [D, HD6], f16, isOutput=False)
    wv_d = nc.declare_dram_parameter("Wv6", [D, HD6], f16, isOutput=False)
    wo_d = nc.declare_dram_parameter("Wo6", [HD6, D], f16, isOutput=False)
    cm_d = nc.declare_dram_parameter("cmneg", [1, NK], f16, isOutput=False)
    out_d = nc.declare_dram_parameter("out", [NQ, D], f16, isOutput=True)

    xT_r = xT_d.rearrange("(c p) n -> p c n", p=P)
    wq_r = wq_d.rearrange("(c p) e -> p c e", p=P)
    wk_r = wk_d.rearrange("(c p) e -> p c e", p=P)
    wv_r = wv_d.rearrange("(c p) e -> p c e", p=P)
    wo_r = wo_d.rearrange("(c p) e -> p c e", p=P)

    # K/Q projection column chunks over NK (PSUM <=512)
    kchunks = [(off, min(512, NK - off)) for off in range(0, NK, 512)]

    with tile.TileContext(nc) as tc:
        with tc.tile_pool(name="persist", bufs=1) as persist:
            kT65 = persist.tile([P, H6, NK], f16)   # rows 0:64 K, row 64 cmneg
            qT65 = persist.tile([P, H6, NK], f16)   # rows 0:64 q/8, row 64 one
            vaug = persist.tile([P, njt, H6, DH + 1], f16)
            attnT = persist.tile([P, 3, NQ], f16)
            wo_sb = persist.tile([P, 3, D], f16)

            pp_cm = tc.tile_pool(name="projsb", bufs=1)
            pp = pp_cm.__enter__()
            pskq_cm = tc.tile_pool(name="pskq_p", bufs=2, space="PSUM")
            pskq_p = pskq_cm.__enter__()

            xT_sb = pp.tile([P, DC, NK], f16)
            wv_sb = pp.tile([P, DC, HD6], f16)
            wk_sb = pp.tile([P, DC, HD6], f16)
            wq_sb = pp.tile([P, DC, HD6], f16)

            # ---- consolidated input DMAs (few, large; Sync queue) ----
            nc.sync.dma_start(out=wv_sb, in_=wv_r)
            xcg = 384
            for cgo in range(0, NK, xcg):
                cgl = min(xcg, NK - cgo)
                nc.sync.dma_start(
                    out=xT_sb[:, :, cgo:cgo + cgl],
                    in_=xT_r[:, :, cgo:cgo + cgl],
                )
            nc.sync.dma_start(out=wk_sb, in_=wk_r)
            nc.sync.dma_start(out=wq_sb, in_=wq_r)
            nc.sync.dma_start(out=wo_sb, in_=wo_r)
            for h in range(H6):
                nc.sync.dma_start(out=kT65[64:65, h, :], in_=cm_d.ap())

            # ones rows/cols early (no deps)
            nc.vector.memset(vaug[:, :, :, DH:DH + 1], 1.0)
            for h in range(H6):
                nc.vector.memset(qT65[64:65, h, :], 1.0)

            def proj_chunk(hdt, w_sb, dest, scale):
                """Project one 128-col chunk of Wk/Wq -> two heads of dest.

                DVE copies map partitions RELATIVELY (in base 64 -> out base
                0 is legal), so the two 64-row halves of the PSUM result go
                straight into dest[0:64, head, :] -- no staging DMA."""
                psks = [
                    pskq_p.tile([P, cs], f32, tag=f"pskq{i}",
                                name=f"pskq{hdt}_{i}")
                    for i, (off, cs) in enumerate(kchunks)
                ]
                for dc in range(DC):
                    for i, (off, cs) in enumerate(kchunks):
                        nc.tensor.matmul(
                            psks[i],
                            w_sb[:, dc, hdt * P:(hdt + 1) * P],
                            xT_sb[:, dc, off:off + cs],
                            start=(dc == 0),
                            stop=(dc == DC - 1),
                        )
                for i, (off, cs) in enumerate(kchunks):
                    for half in range(2):
                        src = psks[i][half * DH:(half + 1) * DH, :]
                        dst = dest[0:DH, 2 * hdt + half, off:off + cs]
                        if scale is None:
                            nc.vector.tensor_copy(dst, src)
                        else:
                            nc.vector.tensor_scalar_mul(
                                dst, in0=src, scalar1=scale
                            )

            # ---------------- projections (sequential) ----------------
            proj_scope = nc.named_scope("proj"); proj_scope.__enter__()
            with tc.tile_pool(name="psv_p", bufs=2, space="PSUM") as psv_p:
                for jt in range(njt):
                    psv = psv_p.tile([P, HD6], f32, tag="psv", name="psv")
                    for dc in range(DC):
                        nc.tensor.matmul(
                            psv,
                            xT_sb[:, dc, jt * P:(jt + 1) * P],
                            wv_sb[:, dc, :],
                            start=(dc == 0),
                            stop=(dc == DC - 1),
                        )
                    nc.vector.tensor_copy(
                        vaug[:, jt, :, 0:DH],
                        psv.rearrange("p (h d) -> p h d", h=H6),
                    )
                for hdt in range(3):
                    proj_chunk(hdt, wk_sb, kT65, None)
                    proj_chunk(hdt, wq_sb, qT65, DH ** -0.5)
            proj_scope.__exit__(None, None, None)

            pskq_cm.__exit__(None, None, None)
            pp_cm.__exit__(None, None, None)

            # attention pools (PSUM: psS 2x2 + psO 2x2 = 8 banks)
            psS_cm = tc.tile_pool(name="psS_p", bufs=2, space="PSUM")
            psS_p = psS_cm.__enter__()
            psO_cm = tc.tile_pool(name="psO_p", bufs=2, space="PSUM")
            psO_p = psO_cm.__enter__()
            pts_cm = tc.tile_pool(name="pts", bufs=3)
            pts = pts_cm.__enter__()
            nrm_cm = tc.tile_pool(name="nrm", bufs=2)
            nrm = nrm_cm.__enter__()

            def attn_head(h, qoff, qlen, glen_max):
                """Attention for head h over queries [qoff, qoff+qlen)."""
                hdt = h // 2
                psO = psO_p.tile([DH + 1, qlen], f32, tag="psO", name="psO")
                G = max(1, glen_max // qlen)
                groups = [
                    list(range(j, min(j + G, njt))) for j in range(0, njt, G)
                ]
                prev = None
                for grp in groups + [None]:
                    if grp is not None:
                        glen = len(grp) * qlen
                        psS = psS_p.tile([P, glen], f32, tag="psS",
                                         name="psS")
                        for gi, jt in enumerate(grp):
                            for so in range(0, qlen, 512):
                                sl = min(512, qlen - so)
                                nc.tensor.matmul(
                                    psS[:, gi * qlen + so:
                                        gi * qlen + so + sl],
                                    kT65[0:DH + 1, h, jt * P:(jt + 1) * P],
                                    qT65[0:DH + 1, h,
                                         qoff + so:qoff + so + sl],
                                    start=True,
                                    stop=True,
                                )
                        pT = pts.tile([P, glen], f16, tag="pT", name="pT")
                        nc.scalar.activation(
                            pT, psS, mybir.ActivationFunctionType.Exp,
                        )
                    if prev is not None:
                        pgrp, ppT = prev
                        for gi, jt in enumerate(pgrp):
                            for so in range(0, qlen, 512):
                                sl = min(512, qlen - so)
                                # start/stop are per PSUM region: each
                                # 512-col half is its own accumulation group
                                nc.tensor.matmul(
                                    psO[:, so:so + sl],
                                    vaug[:, jt, h, :],
                                    ppT[:, gi * qlen + so:
                                        gi * qlen + so + sl],
                                    start=(jt == 0),
                                    stop=(jt == njt - 1),
                                )
                    if grp is not None:
                        prev = (grp, pT)
                # Deferred normalization.  DVE copies map partitions
                # relatively, so: (1) the raw numerator goes straight into
                # attnT (odd heads land on partitions 64:128 directly);
                # (2) the 1-partition s row is refolded into [SRF, qlen/SRF]
                # so the (serial-per-element) DVE reciprocal runs SRF-wide.
                pbase = (h % 2) * DH
                raw = attnT[pbase:pbase + DH, hdt, qoff:qoff + qlen]
                nc.vector.tensor_copy(raw, psO[0:DH, :])
                # DVE partition bases must be 32-aligned, so the s row is
                # refolded onto partitions 0/32/64/96 to make the (serial
                # per element) reciprocal run 4 columns wide.
                SRF = 4 if qlen % 4 == 0 else 1
                qf = qlen // SRF
                srows = nrm.tile([P, qf], f32, tag="sr", name="srows")
                for k in range(SRF):
                    nc.vector.tensor_copy(
                        srows[32 * k:32 * k + 1, :],
                        psO[DH:DH + 1, k * qf:(k + 1) * qf],
                    )
                # one wide reciprocal; lanes with no s data produce unused
                # garbage (rows other than 0/32/64/96)
                rr = nrm.tile([P, qf], f32, tag="rr", name="rr")
                nc.vector.reciprocal(rr, srows)
                rb = nrm.tile([P, qlen], f32, tag="rb", name="rb")
                for k in range(SRF):
                    # partition_broadcast reads the tile's partition 0
                    # regardless of AP base -> bounce row 32k to a base-0
                    # tile first (shifted TensorCopy is legal + works)
                    rk = nrm.tile([1, qf], f32, tag=f"rk{k}", name="rk")
                    nc.vector.tensor_copy(rk, rr[32 * k:32 * k + 1, :])
                    nc.gpsimd.partition_broadcast(
                        rb[:, k * qf:(k + 1) * qf], rk, channels=P,
                    )
                # InstTensorTensor needs all operands at one start partition
                nc.vector.tensor_mul(raw, raw, rb[pbase:pbase + DH, :])

            attn_scope = nc.named_scope("attn"); attn_scope.__enter__()
            for h in range(H6):
                attn_head(h, 0, QM, 1024)
            # tail queries (real queries beyond 1024, if any)
            if nq_tail > 0:
                for h in range(H6):
                    attn_head(h, QM, nq_tail, 1024)
            attn_scope.__exit__(None, None, None)

            nrm_cm.__exit__(None, None, None)
            pts_cm.__exit__(None, None, None)
            psO_cm.__exit__(None, None, None)
            psS_cm.__exit__(None, None, None)

            # ---------------- output projection ----------------
            fin_scope = nc.named_scope("final"); fin_scope.__enter__()
            with tc.tile_pool(name="psF_p", bufs=3, space="PSUM") as psF_p, \
                 tc.tile_pool(name="fin", bufs=3) as fin:
                ftiles = [(o, min(P, NQ - o)) for o in range(0, NQ, P)]
                for (fo, fl) in ftiles:
                    psF = psF_p.tile([fl, D], f32, tag="psF", name="psF")
                    # c-major so both halves share one LDWEIGHTS per c
                    for c in range(3):
                        for half in (slice(0, 512), slice(512, 768)):
                            nc.tensor.matmul(
                                psF[:, half],
                                attnT[:, c, fo:fo + fl],
                                wo_sb[:, c, half],
                                start=(c == 0),
                                stop=(c == 2),
                            )
                    outsb = fin.tile([fl, D], f16, tag="outsb", name="outsb")
                    nc.vector.tensor_copy(outsb, psF)
                    nc.sync.dma_start(
                        out=out_d.ap()[fo:fo + fl, :], in_=outsb
                    )
            fin_scope.__exit__(None, None, None)

    nc.compile()
    _BUILD_CACHE[key] = nc
    return nc


def _marshal(x, x_mask, Wq, Wk, Wv, Wo):
    """Build per-core input maps."""
    x = np.asarray(x, dtype=np.float32)
    x_mask = np.asarray(x_mask).astype(bool)
    Wq = np.asarray(Wq, dtype=np.float32)
    Wk = np.asarray(Wk, dtype=np.float32)
    Wv = np.asarray(Wv, dtype=np.float32)
    Wo = np.asarray(Wo, dtype=np.float32)

    orders = [np.argsort(~x_mask[b], kind="stable") for b in range(B)]
    counts = [int(x_mask[b].sum()) for b in range(B)]
    njt = max(1, -(-max(counts) // P))
    NK = njt * P
    nq_tail = max(0, max(counts) - min(NK, 1024))

    # uniform row for fully-masked queries: (mean_all_rows x @ Wv) @ Wo
    urows = (x.mean(axis=1) @ Wv) @ Wo  # [B, D] f32

    w16 = {
        "Wq": Wq.astype(np.float16),
        "Wk": Wk.astype(np.float16),
        "Wv": Wv.astype(np.float16),
        "Wo": Wo.astype(np.float16),
    }

    in_maps = []
    for c in range(8):
        b, g = c // 2, c % 2
        sel = orders[b][:NK]
        xT16 = np.ascontiguousarray(x[b][sel].T.astype(np.float16))
        cm16 = np.where(x_mask[b][sel], 0.0, MASK_NEG).astype(
            np.float16
        ).reshape(1, NK)
        cols = slice(g * HD6, (g + 1) * HD6)
        in_maps.append({
            "xT": xT16,
            "Wq6": np.ascontiguousarray(w16["Wq"][:, cols]),
            "Wk6": np.ascontiguousarray(w16["Wk"][:, cols]),
            "Wv6": np.ascontiguousarray(w16["Wv"][:, cols]),
            "Wo6": np.ascontiguousarray(w16["Wo"][cols, :]),
            "cmneg": np.ascontiguousarray(cm16),
        })
    return in_maps, njt, nq_tail, orders, counts, urows


def run(x, x_mask, Wq, Wk, Wv, Wo, trace=False, tmpdir=None):
    """Run on 8 cores; returns (full_output, BassKernelResults)."""
    in_maps, njt, nq_tail, orders, counts, urows = _marshal(
        x, x_mask, Wq, Wk, Wv, Wo
    )
    nc = build(njt, nq_tail)
    res = run_bass_kernel_spmd(
        nc, in_maps, core_ids=list(range(8)), trace=trace, tmpdir=tmpdir
    )
    out = np.empty((B, N, D), dtype=np.float32)
    for b in range(B):
        out[b, :, :] = urows[b]
        cnt = counts[b]
        if cnt:
            act = orders[b][:cnt]
            out[b, act] = (
                res.results[2 * b]["out"][:cnt].astype(np.float32)
                + res.results[2 * b + 1]["out"][:cnt].astype(np.float32)
            )
    return out, res


def kernel(**inputs) -> np.ndarray:
    out, _ = run(
        inputs["x"], inputs["x_mask"],
        inputs["Wq"], inputs["Wk"], inputs["Wv"], inputs["Wo"],
        trace=False,
    )
    return out
